# revision 12
# baseline (speedup 1.0000x reference)
"""Trainium2 Bass kernel for nn_ActorCriticLoss (sigmoid probs + lambda returns).

Data-parallel over 8 NeuronCores: batch dim B=131072 is split into 8 shards of
16384 trajectories; each core runs an identical single-core program (no
collectives).

Per-core algorithm:
  - continue_probs = sigmoid(logits)            [Bs, 64]
  - lambda_returns R_t = r_t + GAMMA*c_t*((1-LAMDA)*v_{t+1} + LAMDA*R_{t+1}),
    bootstrap R_{T-1} = v_{T-1}.  Reformulated via W_t = ((1-LAMDA)*v_t +
    LAMDA*R_t)/LAMDA, which satisfies the linear recurrence
        W_t = M_t + b_t * W_{t+1},   b = GAMMA*LAMDA*c,  M = C2*v + r,
        C2 = (1-LAMDA)/LAMDA,  W_{T-1} = v_{T-1}/LAMDA,
    and recovered as R_t = W_t - C2*v_t.  The recurrence maps onto ONE
    hardware tensor_tensor_scan per tile: each SBUF partition holds K whole
    trajectories (64 steps each) concatenated along the free dim (so DMAs are
    fully contiguous); the scan runs ascending in scan-space = descending in
    time via negative-stride views, and b is forced to 0 at each 64-element
    segment start so the carry resets to the per-segment bootstrap, letting a
    single scan instruction sweep all K segments per partition.  The final
    R = W - C2*v op simultaneously un-reverses and compacts 64->63 columns.
"""

import numpy as np

import concourse.bacc as bacc
import concourse.bass as bass
import concourse.mybir as mybir
from concourse import tile
from concourse.bass_utils import run_bass_kernel_spmd

GAMMA = 0.997
LAMDA = 0.95
GL = GAMMA * LAMDA
C2 = (1.0 - LAMDA) / LAMDA

N_CORES = 8
B = 131072
T = 64
BS = B // N_CORES          # 16384 rows per core
TILE_PLAN = [16] * 7 + [8, 8]   # K per tile; sum(K)*128 == BS
KMAX = max(TILE_PLAN)
IO_BUFS = 5
TMP_BUFS = 3

# set by test.py for profiling; harness uses the default
TRACE = False
LAST_RESULT = None


def _build():
    nc = bacc.Bacc(None, target_bir_lowering=False)
    dt = mybir.dt.float32
    logits = nc.dram_tensor("predicted_continue_logits", [BS, T], dt, kind="ExternalInput")
    rewards = nc.dram_tensor("rewards", [BS, T], dt, kind="ExternalInput")
    conts = nc.dram_tensor("continues", [BS, T], dt, kind="ExternalInput")
    values = nc.dram_tensor("critic_values", [BS, T], dt, kind="ExternalInput")
    probs = nc.dram_tensor("probs", [BS, T], dt, kind="ExternalOutput")
    rets = nc.dram_tensor("rets", [BS, T - 1], dt, kind="ExternalOutput")

    def tiled(dram, row_base, kt):
        # [128, kt*T] view, fully contiguous per partition
        return dram[row_base:row_base + 128 * kt, :].rearrange("(p k) t -> p (k t)", p=128)

    with tile.TileContext(nc) as tc:
        with tc.tile_pool(name="io", bufs=IO_BUFS) as io, \
             tc.tile_pool(name="tmp", bufs=TMP_BUFS) as tmp:
            row = 0
            for ti, kt in enumerate(TILE_PLAN):
                f = kt * T
                l_t = io.tile([128, f], dt, tag="l")
                c_t = io.tile([128, f], dt, tag="c")
                v_t = io.tile([128, f], dt, tag="v")
                r_t = io.tile([128, f], dt, tag="r")
                nc.sync.dma_start(c_t[:], tiled(conts, row, kt))
                nc.sync.dma_start(v_t[:], tiled(values, row, kt))
                nc.sync.dma_start(r_t[:], tiled(rewards, row, kt))
                nc.sync.dma_start(l_t[:], tiled(logits, row, kt))

                o1 = io.tile([128, f], dt, tag="o1")
                nc.scalar.activation(o1[:], l_t[:], mybir.ActivationFunctionType.Sigmoid)
                nc.scalar.dma_start(tiled(probs, row, kt), o1[:])

                r3 = r_t[:].rearrange("p (k t) -> p k t", k=kt)
                c3 = c_t[:].rearrange("p (k t) -> p k t", k=kt)
                v3 = v_t[:].rearrange("p (k t) -> p k t", k=kt)

                m_t = tmp.tile([128, f], dt, tag="m")
                b_t = tmp.tile([128, f], dt, tag="b")
                x_t = tmp.tile([128, f], dt, tag="x")
                m3 = m_t[:].rearrange("p (k t) -> p k t", k=kt)
                b3 = b_t[:].rearrange("p (k t) -> p k t", k=kt)
                x3 = x_t[:].rearrange("p (k t) -> p k t", k=kt)

                # scan-space position s in a segment: s=0 carries the bootstrap,
                # s=1..63 correspond to time t=63-s
                nc.vector.tensor_scalar_mul(b3[:, :, 1:64], c3[:, :, 62::-1], GL)
                nc.gpsimd.memset(b3[:, :, 0:1], 0.0)
                # M[s=1..63] = C2*v_t + r_t (reversed); M[s=0] = v[63]/LAMDA
                nc.vector.scalar_tensor_tensor(
                    m3[:, :, 1:64], v3[:, :, 62::-1], C2, r3[:, :, 62::-1],
                    op0=mybir.AluOpType.mult, op1=mybir.AluOpType.add,
                )
                nc.gpsimd.tensor_scalar_mul(m3[:, :, 0:1], v3[:, :, 63:64], 1.0 / LAMDA)

                # W scan: state = b*state + M along the free dim
                nc.vector.tensor_tensor_scan(
                    x_t[:], b_t[:], m_t[:], 0.0,
                    op0=mybir.AluOpType.mult, op1=mybir.AluOpType.add,
                )

                # R_t = W_t - C2*v_t; reads W reversed -> output in natural
                # time order, compacted to 63 columns
                o2 = io.tile([128, kt * (T - 1)], dt, tag="o2")
                o23 = o2[:].rearrange("p (k t) -> p k t", k=kt)
                nc.vector.scalar_tensor_tensor(
                    o23[:, :, 0:63], v3[:, :, 0:63], -C2, x3[:, :, 63:0:-1],
                    op0=mybir.AluOpType.mult, op1=mybir.AluOpType.add,
                )
                nc.scalar.dma_start(tiled(rets, row, kt), o2[:])
                row += 128 * kt
    nc.finalize()
    return nc


_NC = None


def _get_nc():
    global _NC
    if _NC is None:
        _NC = _build()
    return _NC


def kernel(predicted_continue_logits, rewards, continues, critic_values):
    global LAST_RESULT
    nc = _get_nc()
    ins = {
        "predicted_continue_logits": np.asarray(predicted_continue_logits, dtype=np.float32),
        "rewards": np.asarray(rewards, dtype=np.float32),
        "continues": np.asarray(continues, dtype=np.float32),
        "critic_values": np.asarray(critic_values, dtype=np.float32),
    }
    in_maps = [
        {k: np.ascontiguousarray(v[i * BS:(i + 1) * BS]) for k, v in ins.items()}
        for i in range(N_CORES)
    ]
    res = run_bass_kernel_spmd(nc, in_maps, core_ids=list(range(N_CORES)), trace=TRACE)
    LAST_RESULT = res
    probs = np.concatenate([res.results[i]["probs"] for i in range(N_CORES)], axis=0)
    rets = np.concatenate([res.results[i]["rets"] for i in range(N_CORES)], axis=0)
    return probs, rets
